# revision 1
# baseline (speedup 1.0000x reference)
"""CostVolume (9x9 correlation window + leaky_relu) for trn2, 8 NeuronCores.

Problem: x1, x2: [B=8, C=128, H=96, W=320] fp32
  out[b, 9*dy+dx, h, w] =
      leaky_relu(mean_c(x1[b,c,h,w] * x2pad[b,c,h+dy,w+dx]), 0.1)
with x2 zero-padded by 4 on both spatial axes.

Sharding: data-parallel over batch — one batch per core, SPMD over 8 cores.

Per-core kernel design (C=128 lives on SBUF partitions):
  Phase 1: the channel contraction runs on the TensorEngine as band-limited
    matmuls: for each (h row, 128-px w-tile, dy) the 4 32-px strips are
    packed in the 128x128 PE array via tile_position col groups; each strip
    streams its own 40-wide x2 window (psum [128, 9*40]). Eviction fuses
    leaky_relu via scalar_tensor_tensor ((x*0.1) max x; the 1/C mean scale
    is pre-applied to x1) and writes staging dy-minor so the band store to
    a DRAM scratch has contiguous 1440B runs, laid out [h][w][9*j + dy].
  Phase 2: in that scratch layout each pixel's 81 window values are
    contiguous, and the per-pixel diagonal (deskew) is an affine access
    pattern on the DRAM side (where arbitrary strides are legal): gather
    back to SBUF as [w-px partitions, 81], PE-transpose to [81, px], and
    store; the (dx,dy)->d=9*dy+dx permutation is absorbed into the final
    store's access-pattern dims.
"""

from contextlib import ExitStack

import numpy as np

import concourse.bass as bass
import concourse.mybir as mybir
import concourse.tile as tile
from concourse import masks
from concourse.ap import AP
from concourse.bass_utils import run_bass_kernel_spmd

F32 = mybir.dt.float32
C = 128
PAD = 4
D = 9
ND = D * D  # 81

# The walrus build in this toolchain rejects instructions carrying more than
# one sync wait ("Too many sync wait commands"). Split: any instruction with
# >1 on_wait gets preceding same-engine NoOps carrying the extra waits
# (engine streams execute in order, so the gating is identical).
_MAX_WAITS = 1


def _split_sync_waits(nc):
    for bbname, bassbb in nc.bb_map.items():
        bb = bassbb.bb
        insts = list(bb.instructions)
        out = []
        changed = False
        for inst in insts:
            si = getattr(inst, "sync_info", None)
            waits = list(si.on_wait) if (si is not None and si.on_wait) else []
            if len(waits) > _MAX_WAITS:
                changed = True
                spill, keep = waits[:-_MAX_WAITS], waits[-_MAX_WAITS:]
                for k in range(0, len(spill), _MAX_WAITS):
                    chunk = spill[k : k + _MAX_WAITS]
                    out.append(
                        mybir.InstNoOp(
                            name=f"I-waitsplit-{nc.next_id()}",
                            engine=inst.engine,
                            ins=[],
                            outs=[],
                            sync_info=mybir.SyncInfo(on_wait=chunk, on_update=[]),
                        )
                    )
                si.on_wait = keep
            out.append(inst)
        if changed:
            try:
                bb.instructions[:] = out
            except TypeError:
                while len(bb.instructions):
                    bb.instructions.pop()
                for i in out:
                    bb.add_instruction(i)


def _th(t):
    return t.tensor if isinstance(t, AP) else t


def build_kernel(nc: bass.Bass, H=96, W=320, CHUNK=24, HSUB=8):
    assert H % CHUNK == 0 and CHUNK % HSUB == 0 and W % 32 == 0
    x1 = nc.declare_dram_parameter("x1", [C, H, W], F32, isOutput=False)
    x2 = nc.declare_dram_parameter("x2", [C, H, W], F32, isOutput=False)
    out = nc.declare_dram_parameter("out", [ND, H, W], F32, isOutput=True)

    # scratch band: addr = h*(W*360) + w*360 + 9*j + dy   (j in [0,40))
    scratch = nc.dram_tensor("scratch", [H * W * 360], F32)
    sh = _th(scratch)
    oh = _th(out)

    W2 = W + 2 * PAD
    wtiles = []
    wleft, w0 = W, 0
    while wleft > 0:
        mw = min(128, wleft)
        wtiles.append((w0, mw))
        w0 += mw
        wleft -= mw

    # ---------------- phase 1: band matmuls -> scratch ----------------
    with tile.TileContext(nc) as tc, ExitStack() as ctx:
        x_pool = ctx.enter_context(tc.tile_pool(name="x", bufs=1))
        stage_pool = ctx.enter_context(tc.tile_pool(name="stage", bufs=2))
        ps_pool = ctx.enter_context(tc.tile_pool(name="ps", bufs=3, space="PSUM"))

        for ci in range(H // CHUNK):
            h0 = ci * CHUNK
            x1c = x_pool.tile([C, CHUNK, W], F32, tag="x1c")
            nc.sync.dma_start(out=x1c[:], in_=x1[:, h0 : h0 + CHUNK, :])

            x2c = x_pool.tile([C, CHUNK + 8, W2], F32, tag="x2c")
            nc.gpsimd.memset(x2c[:], 0.0)
            lo = max(h0 - PAD, 0)
            hi = min(h0 + CHUNK + PAD, H)
            nc.sync.dma_start(
                out=x2c[:, lo - (h0 - PAD) : hi - (h0 - PAD), PAD : PAD + W],
                in_=x2[:, lo:hi, :],
            )

            for wt, (w0, mw) in enumerate(wtiles):
                nstrip = mw // 32
                for hb in range(CHUNK // HSUB):
                    stg = stage_pool.tile([128, HSUB * 360], F32, tag="stg")
                    sth = _th(stg)
                    relu_tmp = stage_pool.tile([128, 512], F32, tag="relu_tmp")
                    for hh in range(HSUB):
                        h = h0 + hb * HSUB + hh
                        ps = ps_pool.tile([128, 512], F32, tag="band")
                        ph = _th(ps)
                        for s in range(nstrip):
                            wl = w0 + 32 * s
                            for dy in range(D):
                                row = (h - h0) + dy
                                nc.tensor.matmul(
                                    ps[32 * s : 32 * s + 32, 40 * dy : 40 * dy + 40],
                                    x1c[:, h - h0, wl : wl + 32],
                                    x2c[:, row, wl : wl + 40],
                                    start=True,
                                    stop=True,
                                    tile_position=(0, 32 * s),
                                )
                        # fused leaky eviction, dy-minor:
                        # stg[p, hh*360 + 9j + dy] = leaky(ps[p, 40dy + j])
                        ps_ap = AP(ph, 0, [[512, mw], [1, 40], [40, 9]])
                        stg_ap = AP(sth, hh * 360, [[HSUB * 360, mw], [9, 40], [1, 9]])
                        # leaky(x/C) = relu(x*0.9/C) + x*0.1/C  (ACT + DVE)
                        nc.scalar.activation(
                            out=relu_tmp[0:mw, 0:360],
                            in_=AP(ph, 0, [[512, mw], [1, 360]]),
                            func=mybir.ActivationFunctionType.Relu,
                            scale=0.9 / C,
                        )
                        nc.vector.scalar_tensor_tensor(
                            out=stg_ap,
                            in0=ps_ap,
                            scalar=0.1 / C,
                            in1=AP(_th(relu_tmp), 0, [[512, mw], [1, 40], [40, 9]]),
                            op0=mybir.AluOpType.mult,
                            op1=mybir.AluOpType.add,
                        )
                    # band store per strip: contiguous 360-elem runs
                    hbase = h0 + hb * HSUB
                    for s in range(nstrip):
                        src = AP(
                            sth,
                            32 * s * (HSUB * 360),
                            [[HSUB * 360, 32], [360, HSUB], [1, 360]],
                        )
                        dst = AP(
                            sh,
                            hbase * (W * 360) + (w0 + 32 * s) * 360,
                            [[360, 32], [W * 360, HSUB], [1, 360]],
                        )
                        nc.sync.dma_start(out=dst, in_=src)

    # ---------------- phase 2: deskew gather + transpose + store ----------
    with tile.TileContext(nc) as tc, ExitStack() as ctx:
        const_pool = ctx.enter_context(tc.tile_pool(name="const", bufs=1))
        g_pool = ctx.enter_context(tc.tile_pool(name="g", bufs=3))
        o_pool = ctx.enter_context(tc.tile_pool(name="o", bufs=2))
        pst_pool = ctx.enter_context(tc.tile_pool(name="pst", bufs=2, space="PSUM"))

        ident = const_pool.tile([128, 128], F32)
        masks.make_identity(nc, ident[:])

        HB = next(hb for hb in (8, 6, 4, 3, 2, 1) if H % hb == 0)
        for hb in range(H // HB):
            h0 = hb * HB
            gbufs = []
            for wt, (w0, mw) in enumerate(wtiles):
                nstrip = mw // 32
                gbuf = g_pool.tile([128, HB * ND], F32, tag=f"gbuf{wt}")
                gh = _th(gbuf)
                # gbuf[32s+m', hh*81 + 9dx+dy] = scratch[h0+hh, w0+32s+m',
                #                                         9*(m'+dx) + dy]
                for s in range(nstrip):
                    src = AP(
                        sh,
                        h0 * (W * 360) + (w0 + 32 * s) * 360,
                        [[369, 32], [W * 360, HB], [1, ND]],
                    )
                    dst = AP(
                        gh,
                        (32 * s) * (HB * ND),
                        [[HB * ND, 32], [ND, HB], [1, ND]],
                    )
                    nc.sync.dma_start(out=dst, in_=src)
                gbufs.append(gbuf)
            ostg = o_pool.tile([128, HB * W], F32, tag="ostg")
            osh = _th(ostg)
            for hh in range(HB):
                tp = pst_pool.tile([128, 512], F32, tag="tp")
                for wt, (w0, mw) in enumerate(wtiles):
                    nc.tensor.transpose(
                        tp[0:ND, 128 * wt : 128 * wt + mw],
                        gbufs[wt][0:mw, hh * ND : (hh + 1) * ND],
                        ident[0:mw, 0:mw],
                    )
                for wt, (w0, mw) in enumerate(wtiles):
                    nc.vector.tensor_copy(
                        out=ostg[0:ND, hh * W + w0 : hh * W + w0 + mw],
                        in_=AP(_th(tp), 128 * wt, [[512, ND], [1, mw]]),
                    )
            # final stores: one per dx; partitions [9dx, 9dx+9) = dy slice
            for dx in range(D):
                src = AP(
                    osh,
                    (9 * dx) * (HB * W),
                    [[HB * W, D], [W, HB], [1, W]],  # (dy, hh, w)
                )
                dst = AP(
                    oh,
                    dx * (H * W) + h0 * W,
                    [[9 * H * W, D], [W, HB], [1, W]],  # d = 9dy + dx
                )
                nc.sync.dma_start(out=dst, in_=src)

    return nc


_COMPILED = {}


def _build():
    key = "cv"
    if key not in _COMPILED:
        nc = bass.Bass()
        build_kernel(nc)
        _split_sync_waits(nc)
        _COMPILED[key] = nc
    return _COMPILED[key]


def kernel(**inputs) -> np.ndarray:
    x1 = np.asarray(inputs["x1"], dtype=np.float32)
    x2 = np.asarray(inputs["x2"], dtype=np.float32)
    B = x1.shape[0]
    nc = _build()
    core_ids = list(range(8))
    in_maps = [
        {"x1": np.ascontiguousarray(x1[b]), "x2": np.ascontiguousarray(x2[b])}
        for b in range(B)
    ]
    res = run_bass_kernel_spmd(nc, in_maps, core_ids)
    return np.stack([np.asarray(res.results[b]["out"]) for b in range(B)], axis=0)



# revision 10
# speedup vs baseline: 2.7837x; 2.7837x over previous
"""CostVolume (9x9 correlation window + leaky_relu) for trn2, 8 NeuronCores.

Problem: x1, x2: [B=8, C=128, H=96, W=320] fp32
  out[b, 9*dy+dx, h, w] =
      leaky_relu(mean_c(x1[b,c,h,w] * x2pad[b,c,h+dy,w+dx]), 0.1)
with x2 zero-padded by 4 on both spatial axes.

Sharding: data-parallel over batch - one batch per core, SPMD over 8 cores.

Per-core pipeline (single TileContext, h-chunk pipelined):
  - inputs stream HBM->SBUF via gpsimd cast-DMA straight to bf16 (the cast is
    free in the SDMA datapath; fp32 never lands in SBUF).
  - channel contraction on TensorE as bf16 band matmuls: per (h, 32-px strip)
    one matmul with the 9 dy-window rows batched into a single 360-wide moving
    AP; 4 strips pack the 128x128 PE array via tile_position col groups into
    one PSUM bank [128, 360].
  - eviction on DVE: one scalar_tensor_tensor max(0.1*x, x) (= unscaled leaky)
    converting PSUM fp32 -> SBUF fp16, relayouting dy-minor so each pixel's
    window values are contiguous (t = 9*j + dy).
  - deskew (extract each pixel's 81 diagonal values from the band) via a DRAM
    fp16 scratch: band store is contiguous; the per-pixel diagonal is an
    affine gather on the DRAM side (stride 369 = 360+9), batched over 8 h rows.
  - PE transposes [px, 81] -> [81, px] (fp16 identity), ACT copy applies the
    1/128 mean scale and converts to fp32, one big store per 24-row chunk with
    the (dx,dy)->d=9*dy+dx permutation absorbed in the store AP.
"""

from contextlib import ExitStack

import numpy as np

import concourse.bass as bass
import concourse.mybir as mybir
import concourse.tile as tile
from concourse import masks
from concourse.ap import AP
from concourse.bass_utils import run_bass_kernel_spmd

F32 = mybir.dt.float32
F16 = mybir.dt.float16
BF16 = mybir.dt.bfloat16
C = 128
PAD = 4
D = 9
ND = D * D  # 81
WIN = 40  # 32 + 8: moving window per strip
BAND = D * WIN  # 360 band values per pixel in scratch

# The walrus build in this toolchain rejects instructions carrying more than
# one sync wait ("Too many sync wait commands"). Split: any instruction with
# >1 on_wait gets preceding same-engine NoOps carrying the extra waits
# (engine streams execute in order, so the gating is identical).
_MAX_WAITS = 1


def _split_sync_waits(nc):
    for bbname, bassbb in nc.bb_map.items():
        bb = bassbb.bb
        insts = list(bb.instructions)
        out = []
        changed = False
        for inst in insts:
            si = getattr(inst, "sync_info", None)
            waits = list(si.on_wait) if (si is not None and si.on_wait) else []
            if len(waits) > _MAX_WAITS:
                changed = True
                spill, keep = waits[:-_MAX_WAITS], waits[-_MAX_WAITS:]
                for k in range(0, len(spill), _MAX_WAITS):
                    chunk = spill[k : k + _MAX_WAITS]
                    out.append(
                        mybir.InstNoOp(
                            name=f"I-waitsplit-{nc.next_id()}",
                            engine=inst.engine,
                            ins=[],
                            outs=[],
                            sync_info=mybir.SyncInfo(on_wait=chunk, on_update=[]),
                        )
                    )
                si.on_wait = keep
            out.append(inst)
        if changed:
            try:
                bb.instructions[:] = out
            except TypeError:
                while len(bb.instructions):
                    bb.instructions.pop()
                for i in out:
                    bb.add_instruction(i)


def _th(t):
    return t.tensor if isinstance(t, AP) else t


def build_kernel(nc: bass.Bass, H=96, W=320, CHUNK=24, HB=12):
    assert H % CHUNK == 0 and CHUNK % HB == 0
    x1 = nc.declare_dram_parameter("x1", [C, H, W], F32, isOutput=False)
    x2 = nc.declare_dram_parameter("x2", [C, H, W], F32, isOutput=False)
    out = nc.declare_dram_parameter("out", [ND, H, W], F32, isOutput=True)

    # fp16 band scratch: addr = (h*W + w)*BAND + 9*j + dy  (j in [0,40))
    scratch = nc.dram_tensor("scratch", [H * W * BAND], F16)
    sh = _th(scratch)
    oh = _th(out)

    W2 = W + 2 * PAD
    HC = CHUNK + 2 * PAD
    # (w0, mw): 128/128/64 pixel column tiles; each holds mw//32 strips
    wtiles = []
    wleft, w0 = W, 0
    while wleft > 0:
        mw = min(128, wleft)
        wtiles.append((w0, mw))
        w0 += mw
        wleft -= mw
    nchunks = H // CHUNK
    nblocks = CHUNK // HB

    with tile.TileContext(nc) as tc, ExitStack() as ctx:
        const_pool = ctx.enter_context(tc.tile_pool(name="const", bufs=1))
        x_pool = ctx.enter_context(tc.tile_pool(name="x", bufs=2))
        stg_pool = ctx.enter_context(tc.tile_pool(name="stg", bufs=4))
        rl_pool = ctx.enter_context(tc.tile_pool(name="rl", bufs=3))
        g_pool = ctx.enter_context(tc.tile_pool(name="g", bufs=2))
        o_pool = ctx.enter_context(tc.tile_pool(name="o", bufs=2))
        ps_pool = ctx.enter_context(tc.tile_pool(name="ps", bufs=4, space="PSUM"))
        tp_pool = ctx.enter_context(tc.tile_pool(name="tp", bufs=3, space="PSUM"))

        ident = const_pool.tile([128, 128], F16)
        masks.make_identity(nc, ident[:])
        idh = _th(ident)

        # deferred per-block state: (ci, b, gbufs, h0b)
        pending = None
        ostg_by_chunk = {}

        def flush_block(blk):
            ci, b, gbufs, h0b = blk
            ostg, _ = ostg_by_chunk[ci]
            osh = _th(ostg)
            for hh in range(HB):
                tp = tp_pool.tile([128, W], F16, tag="tp")
                tph = _th(tp)
                for wt, (w0, mw) in enumerate(wtiles):
                    gh = _th(gbufs[wt])
                    nc.tensor.transpose(
                        AP(tph, w0, [[W, ND], [1, mw]]),
                        AP(gh, hh * ND, [[HB * ND, mw], [1, ND]]),
                        AP(idh, 0, [[128, mw], [1, mw]]),
                    )
                hl = (h0b - ci * CHUNK) + hh  # row within the chunk
                nc.scalar.activation(
                    out=AP(osh, hl * W, [[CHUNK * W, ND], [1, W]]),
                    in_=AP(tph, 0, [[W, ND], [1, W]]),
                    func=mybir.ActivationFunctionType.Copy,
                    scale=1.0 / C,
                )
            ostg_by_chunk[ci][1].append(b)
            if len(ostg_by_chunk[ci][1]) == nblocks:
                # chunk complete: one store, (dx,dy) -> d=9*dy+dx via AP dims
                osh = _th(ostg_by_chunk[ci][0])
                nc.sync.dma_start(
                    out=AP(
                        oh,
                        ci * CHUNK * W,
                        [[H * W, D], [D * H * W, D], [1, CHUNK * W]],
                    ),
                    in_=AP(osh, 0, [[CHUNK * W, ND], [1, CHUNK * W]]),
                )
                del ostg_by_chunk[ci]

        for ci in range(nchunks):
            h0 = ci * CHUNK
            x1c = x_pool.tile([C, CHUNK * W], BF16, tag="x1c")
            x1h = _th(x1c)
            nc.gpsimd.dma_start(
                out=x1c[:],
                in_=AP(_th(x1), h0 * W, [[H * W, C], [1, CHUNK * W]]),
            )

            x2c = x_pool.tile([C, HC * W2], BF16, tag="x2c")
            x2h = _th(x2c)
            # zero the W-pad columns (4 left + 4 right per row)
            nc.gpsimd.memset(AP(x2h, 0, [[HC * W2, C], [W2, HC], [1, PAD]]), 0.0)
            nc.gpsimd.memset(
                AP(x2h, PAD + W, [[HC * W2, C], [W2, HC], [1, PAD]]), 0.0
            )
            lo = max(h0 - PAD, 0)
            hi = min(h0 + CHUNK + PAD, H)
            if lo > h0 - PAD:  # first chunk: zero top halo rows
                nc.gpsimd.memset(
                    AP(x2h, PAD, [[HC * W2, C], [W2, lo - (h0 - PAD)], [1, W]]), 0.0
                )
            if hi < h0 + CHUNK + PAD:  # last chunk: zero bottom halo rows
                r0 = hi - (h0 - PAD)
                nc.gpsimd.memset(
                    AP(x2h, r0 * W2 + PAD, [[HC * W2, C], [W2, HC - r0], [1, W]]),
                    0.0,
                )
            nc.gpsimd.dma_start(
                out=AP(
                    x2h,
                    (lo - (h0 - PAD)) * W2 + PAD,
                    [[HC * W2, C], [W2, hi - lo], [1, W]],
                ),
                in_=AP(_th(x2), lo * W, [[H * W, C], [1, (hi - lo) * W]]),
            )

            ostg_by_chunk[ci] = [
                o_pool.tile([128, CHUNK * W], F32, tag="ostg", name="ostg"),
                [],
            ]

            for b in range(nblocks):
                h0b = h0 + b * HB
                # per-(block, wtile) fp16 band staging: [pixel, hh*360 + 9j+dy]
                stgs = [
                    stg_pool.tile([128, HB * BAND], F16, tag="stg", name=f"stg{wt}")
                    for wt in range(len(wtiles))
                ]
                for hh in range(HB):
                    h = h0b + hh
                    r = h - h0  # x1 row within chunk; x2 row base = r (top halo)
                    for wt, (w0, mw) in enumerate(wtiles):
                        nstrip = mw // 32
                        ps = ps_pool.tile([128, 512], F32, tag="band")
                        ph = _th(ps)
                        for s in range(nstrip):
                            wl = w0 + 32 * s
                            nc.tensor.matmul(
                                AP(ph, 32 * s * 512, [[512, 32], [1, BAND]]),
                                AP(x1h, r * W + wl, [[CHUNK * W, C], [1, 32]]),
                                AP(x2h, r * W2 + wl, [[HC * W2, C], [W2, D], [1, WIN]]),
                                start=True,
                                stop=True,
                                tile_position=(0, 32 * s),
                            )
                        # leaky eviction fp32 psum -> fp16 band, dy-minor:
                        # leaky(x) = relu(0.9x) + 0.1x  (ACT + DVE, one PSUM
                        # read each); stg[p, hh*360+9j+dy] = leaky(ps[p,40dy+j])
                        sth = _th(stgs[wt])
                        relu_t = rl_pool.tile([128, BAND], F16, tag="relu_t")
                        rth = _th(relu_t)
                        nc.scalar.activation(
                            out=AP(rth, 0, [[BAND, mw], [1, BAND]]),
                            in_=AP(ph, 0, [[512, mw], [1, BAND]]),
                            func=mybir.ActivationFunctionType.Relu,
                            scale=0.9,
                        )
                        nc.vector.scalar_tensor_tensor(
                            out=AP(
                                sth, hh * BAND, [[HB * BAND, mw], [D, WIN], [1, D]]
                            ),
                            in0=AP(ph, 0, [[512, mw], [1, WIN], [WIN, D]]),
                            scalar=0.1,
                            in1=AP(rth, 0, [[BAND, mw], [1, WIN], [WIN, D]]),
                            op0=mybir.AluOpType.mult,
                            op1=mybir.AluOpType.add,
                        )
                # band stores: one per wtile, contiguous 720B runs
                for wt, (w0, mw) in enumerate(wtiles):
                    sth = _th(stgs[wt])
                    nc.sync.dma_start(
                        out=AP(
                            sh,
                            (h0b * W + w0) * BAND,
                            [[BAND, mw], [W * BAND, HB], [1, BAND]],
                        ),
                        in_=AP(
                            sth, 0, [[HB * BAND, mw], [BAND, HB], [1, BAND]]
                        ),
                    )
                # gathers for this block: per strip, 8 h rows, diagonal on DRAM
                gbufs = []
                for wt, (w0, mw) in enumerate(wtiles):
                    gbuf = g_pool.tile([128, HB * ND], F16, tag=f"gb{wt}")
                    gh = _th(gbuf)
                    for s in range(mw // 32):
                        wl = w0 + 32 * s
                        nc.sync.dma_start(
                            out=AP(
                                gh,
                                (32 * s) * (HB * ND),
                                [[HB * ND, 32], [ND, HB], [1, ND]],
                            ),
                            in_=AP(
                                sh,
                                (h0b * W + wl) * BAND,
                                [[BAND + D, 32], [W * BAND, HB], [1, ND]],
                            ),
                        )
                    gbufs.append(gbuf)
                if pending is not None:
                    flush_block(pending)
                pending = (ci, b, gbufs, h0b)

        flush_block(pending)

    return nc


_COMPILED = {}


def _build():
    key = "cv"
    if key not in _COMPILED:
        nc = bass.Bass()
        build_kernel(nc)
        _split_sync_waits(nc)
        _COMPILED[key] = nc
    return _COMPILED[key]


def kernel(**inputs) -> np.ndarray:
    x1 = np.asarray(inputs["x1"], dtype=np.float32)
    x2 = np.asarray(inputs["x2"], dtype=np.float32)
    B = x1.shape[0]
    nc = _build()
    core_ids = list(range(8))
    in_maps = [
        {"x1": np.ascontiguousarray(x1[b]), "x2": np.ascontiguousarray(x2[b])}
        for b in range(B)
    ]
    res = run_bass_kernel_spmd(nc, in_maps, core_ids)
    return np.stack([np.asarray(res.results[b]["out"]) for b in range(B)], axis=0)


# revision 19
# speedup vs baseline: 3.1713x; 1.1393x over previous
"""CostVolume (9x9 correlation window + leaky_relu) for trn2, 8 NeuronCores.

Problem: x1, x2: [B=8, C=128, H=96, W=320] fp32
  out[b, 9*dy+dx, h, w] =
      leaky_relu(mean_c(x1[b,c,h,w] * x2pad[b,c,h+dy,w+dx]), 0.1)
with x2 zero-padded by 4 on both spatial axes.

Sharding: data-parallel over batch - one batch per core, SPMD over 8 cores.

Per-core pipeline (single TileContext, h-chunk pipelined):
  - inputs stream HBM->SBUF via gpsimd cast-DMA straight to bf16 (the cast is
    free in the SDMA datapath; fp32 never lands in SBUF).
  - channel contraction on TensorE as bf16 band matmuls: per (h, 32-px strip)
    one matmul with the 9 dy-window rows batched into a single 360-wide moving
    AP; 4 strips pack the 128x128 PE array via tile_position col groups into
    one PSUM bank [128, 360].
  - eviction on DVE: one scalar_tensor_tensor max(0.1*x, x) (= unscaled leaky)
    converting PSUM fp32 -> SBUF fp16, relayouting dy-minor so each pixel's
    window values are contiguous (t = 9*j + dy).
  - deskew (extract each pixel's 81 diagonal values from the band) via a DRAM
    fp16 scratch: band store is contiguous; the per-pixel diagonal is an
    affine gather on the DRAM side (stride 369 = 360+9), batched over 8 h rows.
  - PE transposes [px, 81] -> [81, px] (fp16 identity), ACT copy applies the
    1/128 mean scale and converts to fp32, one big store per 24-row chunk with
    the (dx,dy)->d=9*dy+dx permutation absorbed in the store AP.
"""

from contextlib import ExitStack

import numpy as np

import concourse.bass as bass
import concourse.mybir as mybir
import concourse.tile as tile
from concourse import masks
from concourse.ap import AP
from concourse.bass_utils import run_bass_kernel_spmd

F32 = mybir.dt.float32
F16 = mybir.dt.float16
BF16 = mybir.dt.bfloat16
C = 128
PAD = 4
D = 9
ND = D * D  # 81
WIN = 40  # 32 + 8: moving window per strip
BAND = D * WIN  # 360 band values per pixel in scratch

# The walrus build in this toolchain rejects instructions carrying more than
# one sync wait ("Too many sync wait commands"). Split: any instruction with
# >1 on_wait gets preceding same-engine NoOps carrying the extra waits
# (engine streams execute in order, so the gating is identical).
_MAX_WAITS = 1


def _split_sync_waits(nc):
    for bbname, bassbb in nc.bb_map.items():
        bb = bassbb.bb
        insts = list(bb.instructions)
        out = []
        changed = False
        for inst in insts:
            si = getattr(inst, "sync_info", None)
            waits = list(si.on_wait) if (si is not None and si.on_wait) else []
            if len(waits) > _MAX_WAITS:
                changed = True
                spill, keep = waits[:-_MAX_WAITS], waits[-_MAX_WAITS:]
                for k in range(0, len(spill), _MAX_WAITS):
                    chunk = spill[k : k + _MAX_WAITS]
                    out.append(
                        mybir.InstNoOp(
                            name=f"I-waitsplit-{nc.next_id()}",
                            engine=inst.engine,
                            ins=[],
                            outs=[],
                            sync_info=mybir.SyncInfo(on_wait=chunk, on_update=[]),
                        )
                    )
                si.on_wait = keep
            out.append(inst)
        if changed:
            try:
                bb.instructions[:] = out
            except TypeError:
                while len(bb.instructions):
                    bb.instructions.pop()
                for i in out:
                    bb.add_instruction(i)


def _th(t):
    return t.tensor if isinstance(t, AP) else t


def build_kernel(nc: bass.Bass, H=96, W=320, CHUNK=24, HB=12):
    assert H % CHUNK == 0 and CHUNK % HB == 0
    x1 = nc.declare_dram_parameter("x1", [C, H, W], F32, isOutput=False)
    x2 = nc.declare_dram_parameter("x2", [C, H, W], F32, isOutput=False)
    out = nc.declare_dram_parameter("out", [ND, H, W], F32, isOutput=True)

    # fp16 band scratch, w-major: addr = (w*H + h)*BAND + 9*j + dy
    # (w-major makes the band store one contiguous 8640B run per pixel-block)
    scratch = nc.dram_tensor("scratch", [H * W * BAND], F16)
    sh = _th(scratch)
    oh = _th(out)

    W2 = W + 2 * PAD
    HC = CHUNK + 2 * PAD
    # (w0, mw): 128/128/64 pixel column tiles; each holds mw//32 strips
    wtiles = []
    wleft, w0 = W, 0
    while wleft > 0:
        mw = min(128, wleft)
        wtiles.append((w0, mw))
        w0 += mw
        wleft -= mw
    nchunks = H // CHUNK
    nblocks = CHUNK // HB

    with tile.TileContext(nc) as tc, ExitStack() as ctx:
        const_pool = ctx.enter_context(tc.tile_pool(name="const", bufs=1))
        x_pool = ctx.enter_context(tc.tile_pool(name="x", bufs=2))
        stg_pool = ctx.enter_context(tc.tile_pool(name="stg", bufs=4))
        g_pool = ctx.enter_context(tc.tile_pool(name="g", bufs=2))
        o_pool = ctx.enter_context(tc.tile_pool(name="o", bufs=2))
        ps_pool = ctx.enter_context(tc.tile_pool(name="ps", bufs=4, space="PSUM"))
        tp_pool = ctx.enter_context(tc.tile_pool(name="tp", bufs=3, space="PSUM"))

        ident = const_pool.tile([128, 128], F16)
        masks.make_identity(nc, ident[:])
        idh = _th(ident)

        # deferred per-block state: (ci, b, gbufs, h0b)
        pending = None
        ostg_by_chunk = {}

        def flush_block(blk):
            ci, b, gbufs, h0b = blk
            ostg, _ = ostg_by_chunk[ci]
            osh = _th(ostg)
            for hh in range(HB):
                tp = tp_pool.tile([128, W], F16, tag="tp")
                tph = _th(tp)
                for wt, (w0, mw) in enumerate(wtiles):
                    gh = _th(gbufs[wt])
                    nc.tensor.transpose(
                        AP(tph, w0, [[W, ND], [1, mw]]),
                        AP(gh, hh * ND, [[HB * ND, mw], [1, ND]]),
                        AP(idh, 0, [[128, mw], [1, mw]]),
                    )
                hl = (h0b - ci * CHUNK) + hh  # row within the chunk
                nc.vector.tensor_copy(
                    out=AP(osh, hl * W, [[CHUNK * W, ND], [1, W]]),
                    in_=AP(tph, 0, [[W, ND], [1, W]]),
                )
            ostg_by_chunk[ci][1].append(b)
            if len(ostg_by_chunk[ci][1]) == nblocks:
                # chunk complete: one store, (dx,dy) -> d=9*dy+dx via AP dims
                osh = _th(ostg_by_chunk[ci][0])
                nc.sync.dma_start(
                    out=AP(
                        oh,
                        ci * CHUNK * W,
                        [[H * W, D], [D * H * W, D], [1, CHUNK * W]],
                    ),
                    in_=AP(osh, 0, [[CHUNK * W, ND], [1, CHUNK * W]]),
                )
                del ostg_by_chunk[ci]

        def emit_loads(ci):
            h0 = ci * CHUNK
            x1c = x_pool.tile([C, CHUNK * W], BF16, tag="x1c", name="x1c")
            x1h = _th(x1c)
            nc.gpsimd.dma_start(
                out=x1c[:],
                in_=AP(_th(x1), h0 * W, [[H * W, C], [1, CHUNK * W]]),
            )

            x2c = x_pool.tile([C, HC * W2], BF16, tag="x2c", name="x2c")
            x2h = _th(x2c)
            # zero the W-pad columns (4 left + 4 right per row)
            nc.gpsimd.memset(AP(x2h, 0, [[HC * W2, C], [W2, HC], [1, PAD]]), 0.0)
            nc.gpsimd.memset(
                AP(x2h, PAD + W, [[HC * W2, C], [W2, HC], [1, PAD]]), 0.0
            )
            lo = max(h0 - PAD, 0)
            hi = min(h0 + CHUNK + PAD, H)
            if lo > h0 - PAD:  # first chunk: zero top halo rows
                nc.gpsimd.memset(
                    AP(x2h, PAD, [[HC * W2, C], [W2, lo - (h0 - PAD)], [1, W]]), 0.0
                )
            if hi < h0 + CHUNK + PAD:  # last chunk: zero bottom halo rows
                r0 = hi - (h0 - PAD)
                nc.gpsimd.memset(
                    AP(x2h, r0 * W2 + PAD, [[HC * W2, C], [W2, HC - r0], [1, W]]),
                    0.0,
                )
            nc.gpsimd.dma_start(
                out=AP(
                    x2h,
                    (lo - (h0 - PAD)) * W2 + PAD,
                    [[HC * W2, C], [W2, hi - lo], [1, W]],
                ),
                in_=AP(_th(x2), lo * W, [[H * W, C], [1, (hi - lo) * W]]),
            )
            return x1h, x2h

        loaded = {0: emit_loads(0)}

        for ci in range(nchunks):
            h0 = ci * CHUNK
            x1h, x2h = loaded.pop(ci)

            ostg_by_chunk[ci] = [
                o_pool.tile([128, CHUNK * W], F32, tag="ostg", name="ostg"),
                [],
            ]

            for b in range(nblocks):
                h0b = h0 + b * HB
                # per-(block, wtile) fp16 band staging: [pixel, hh*360 + 9j+dy]
                stgs = [
                    stg_pool.tile([128, HB * BAND], F16, tag="stg", name=f"stg{wt}")
                    for wt in range(len(wtiles))
                ]
                for hh in range(HB):
                    h = h0b + hh
                    r = h - h0  # x1 row within chunk; x2 row base = r (top halo)
                    for wt, (w0, mw) in enumerate(wtiles):
                        nstrip = mw // 32
                        ps = ps_pool.tile([128, 512], F32, tag="band")
                        ph = _th(ps)
                        for s in range(nstrip):
                            wl = w0 + 32 * s
                            # moving dims (j outer, dy inner) so psum lands
                            # already dy-minor: ps[m, 9j+dy]
                            nc.tensor.matmul(
                                AP(ph, 32 * s * 512, [[512, 32], [1, BAND]]),
                                AP(x1h, r * W + wl, [[CHUNK * W, C], [1, 32]]),
                                AP(x2h, r * W2 + wl, [[HC * W2, C], [1, WIN], [W2, D]]),
                                start=True,
                                stop=True,
                                tile_position=(0, 32 * s),
                            )
                        # leaky eviction fp32 psum -> fp16 band, contiguous,
                        # one ACT op: stg[p, hh*360+t] = leaky_0.1(ps[p,t]/C)
                        sth = _th(stgs[wt])
                        nc.scalar.activation(
                            out=AP(
                                sth, hh * BAND, [[HB * BAND, mw], [1, BAND]]
                            ),
                            in_=AP(ph, 0, [[512, mw], [1, BAND]]),
                            func=mybir.ActivationFunctionType.Prelu,
                            scale=1.0 / C,
                            alpha=0.1,
                        )
                # band stores: one per wtile, contiguous 8640B per pixel
                for wt, (w0, mw) in enumerate(wtiles):
                    sth = _th(stgs[wt])
                    nc.sync.dma_start(
                        out=AP(
                            sh,
                            (w0 * H + h0b) * BAND,
                            [[H * BAND, mw], [1, HB * BAND]],
                        ),
                        in_=AP(sth, 0, [[HB * BAND, mw], [1, HB * BAND]]),
                    )
                # gathers for this block: per strip, HB h rows, diagonal on
                # DRAM (stride H*BAND+9); issue alternates HWDGE rings
                gbufs = []
                sg = 0
                for wt, (w0, mw) in enumerate(wtiles):
                    gbuf = g_pool.tile([128, HB * ND], F16, tag=f"gb{wt}")
                    gh = _th(gbuf)
                    for s in range(mw // 32):
                        wl = w0 + 32 * s
                        eng = nc.sync if sg % 2 == 0 else nc.scalar
                        sg += 1
                        eng.dma_start(
                            out=AP(
                                gh,
                                (32 * s) * (HB * ND),
                                [[HB * ND, 32], [ND, HB], [1, ND]],
                            ),
                            in_=AP(
                                sh,
                                (wl * H + h0b) * BAND,
                                [[H * BAND + D, 32], [BAND, HB], [1, ND]],
                            ),
                        )
                    gbufs.append(gbuf)
                if b == 0 and ci + 1 < nchunks:
                    # prefetch next chunk's inputs while this one computes
                    loaded[ci + 1] = emit_loads(ci + 1)
                if pending is not None:
                    flush_block(pending)
                pending = (ci, b, gbufs, h0b)

        flush_block(pending)

    return nc


_COMPILED = {}


def _build():
    key = "cv"
    if key not in _COMPILED:
        nc = bass.Bass()
        build_kernel(nc)
        _split_sync_waits(nc)
        _COMPILED[key] = nc
    return _COMPILED[key]


def kernel(**inputs) -> np.ndarray:
    x1 = np.asarray(inputs["x1"], dtype=np.float32)
    x2 = np.asarray(inputs["x2"], dtype=np.float32)
    B = x1.shape[0]
    nc = _build()
    core_ids = list(range(8))
    in_maps = [
        {"x1": np.ascontiguousarray(x1[b]), "x2": np.ascontiguousarray(x2[b])}
        for b in range(B)
    ]
    res = run_bass_kernel_spmd(nc, in_maps, core_ids)
    return np.stack([np.asarray(res.results[b]["out"]) for b in range(B)], axis=0)


# revision 20
# speedup vs baseline: 3.4917x; 1.1010x over previous
"""CostVolume (9x9 correlation window + leaky_relu) for trn2, 8 NeuronCores.

Problem: x1, x2: [B=8, C=128, H=96, W=320] fp32
  out[b, 9*dy+dx, h, w] =
      leaky_relu(mean_c(x1[b,c,h,w] * x2pad[b,c,h+dy,w+dx]), 0.1)
with x2 zero-padded by 4 on both spatial axes.

Sharding: data-parallel over batch - one batch per core, SPMD over 8 cores.

Per-core pipeline (single TileContext, h-chunk pipelined):
  - inputs stream HBM->SBUF via gpsimd cast-DMA straight to bf16 (the cast is
    free in the SDMA datapath; fp32 never lands in SBUF).
  - channel contraction on TensorE as bf16 band matmuls: per (h, 32-px strip)
    one matmul with the 9 dy-window rows batched into a single 360-wide moving
    AP; 4 strips pack the 128x128 PE array via tile_position col groups into
    one PSUM bank [128, 360].
  - eviction on DVE: one scalar_tensor_tensor max(0.1*x, x) (= unscaled leaky)
    converting PSUM fp32 -> SBUF fp16, relayouting dy-minor so each pixel's
    window values are contiguous (t = 9*j + dy).
  - deskew (extract each pixel's 81 diagonal values from the band) via a DRAM
    fp16 scratch: band store is contiguous; the per-pixel diagonal is an
    affine gather on the DRAM side (stride 369 = 360+9), batched over 8 h rows.
  - PE transposes [px, 81] -> [81, px] (fp16 identity), ACT copy applies the
    1/128 mean scale and converts to fp32, one big store per 24-row chunk with
    the (dx,dy)->d=9*dy+dx permutation absorbed in the store AP.
"""

from contextlib import ExitStack

import numpy as np

import concourse.bass as bass
import concourse.mybir as mybir
import concourse.tile as tile
from concourse import masks
from concourse.ap import AP
from concourse.bass_utils import run_bass_kernel_spmd

F32 = mybir.dt.float32
F16 = mybir.dt.float16
BF16 = mybir.dt.bfloat16
C = 128
PAD = 4
D = 9
ND = D * D  # 81
WIN = 40  # 32 + 8: moving window per strip
BAND = D * WIN  # 360 band values per pixel in scratch

# The walrus build in this toolchain rejects instructions carrying more than
# one sync wait ("Too many sync wait commands"). Split: any instruction with
# >1 on_wait gets preceding same-engine NoOps carrying the extra waits
# (engine streams execute in order, so the gating is identical).
_MAX_WAITS = 1


def _split_sync_waits(nc):
    for bbname, bassbb in nc.bb_map.items():
        bb = bassbb.bb
        insts = list(bb.instructions)
        out = []
        changed = False
        for inst in insts:
            si = getattr(inst, "sync_info", None)
            waits = list(si.on_wait) if (si is not None and si.on_wait) else []
            if len(waits) > _MAX_WAITS:
                changed = True
                spill, keep = waits[:-_MAX_WAITS], waits[-_MAX_WAITS:]
                for k in range(0, len(spill), _MAX_WAITS):
                    chunk = spill[k : k + _MAX_WAITS]
                    out.append(
                        mybir.InstNoOp(
                            name=f"I-waitsplit-{nc.next_id()}",
                            engine=inst.engine,
                            ins=[],
                            outs=[],
                            sync_info=mybir.SyncInfo(on_wait=chunk, on_update=[]),
                        )
                    )
                si.on_wait = keep
            out.append(inst)
        if changed:
            try:
                bb.instructions[:] = out
            except TypeError:
                while len(bb.instructions):
                    bb.instructions.pop()
                for i in out:
                    bb.add_instruction(i)


def _th(t):
    return t.tensor if isinstance(t, AP) else t


def build_kernel(nc: bass.Bass, H=96, W=320, CHUNK=24, HB=12):
    assert H % CHUNK == 0 and CHUNK % HB == 0
    x1 = nc.declare_dram_parameter("x1", [C, H, W], F32, isOutput=False)
    x2 = nc.declare_dram_parameter("x2", [C, H, W], F32, isOutput=False)
    out = nc.declare_dram_parameter("out", [ND, H, W], F32, isOutput=True)

    # fp16 band scratch, w-major: addr = (w*H + h)*BAND + 9*j + dy
    # (w-major makes the band store one contiguous 8640B run per pixel-block)
    scratch = nc.dram_tensor("scratch", [H * W * BAND], F16)
    sh = _th(scratch)
    oh = _th(out)

    W2 = W + 2 * PAD
    HC = CHUNK + 2 * PAD
    # (w0, mw): 128/128/64 pixel column tiles; each holds mw//32 strips
    wtiles = []
    wleft, w0 = W, 0
    while wleft > 0:
        mw = min(128, wleft)
        wtiles.append((w0, mw))
        w0 += mw
        wleft -= mw
    nchunks = H // CHUNK
    nblocks = CHUNK // HB

    with tile.TileContext(nc) as tc, ExitStack() as ctx:
        const_pool = ctx.enter_context(tc.tile_pool(name="const", bufs=1))
        x_pool = ctx.enter_context(tc.tile_pool(name="x", bufs=2))
        stg_pool = ctx.enter_context(tc.tile_pool(name="stg", bufs=4))
        g_pool = ctx.enter_context(tc.tile_pool(name="g", bufs=2))
        o_pool = ctx.enter_context(tc.tile_pool(name="o", bufs=2))
        ps_pool = ctx.enter_context(tc.tile_pool(name="ps", bufs=4, space="PSUM"))
        tp_pool = ctx.enter_context(tc.tile_pool(name="tp", bufs=3, space="PSUM"))

        ident = const_pool.tile([128, 128], F16)
        masks.make_identity(nc, ident[:])
        idh = _th(ident)

        # deferred per-block state: (ci, b, gbufs, h0b)
        pending = None
        ostg_by_chunk = {}

        def flush_block(blk):
            ci, b, gbufs, h0b = blk
            ostg, _ = ostg_by_chunk[ci]
            osh = _th(ostg)
            for hh in range(HB):
                tp = tp_pool.tile([128, W], F16, tag="tp")
                tph = _th(tp)
                for wt, (w0, mw) in enumerate(wtiles):
                    gh = _th(gbufs[wt])
                    nc.tensor.transpose(
                        AP(tph, w0, [[W, ND], [1, mw]]),
                        AP(gh, hh * ND, [[HB * ND, mw], [1, ND]]),
                        AP(idh, 0, [[128, mw], [1, mw]]),
                    )
                hl = (h0b - ci * CHUNK) + hh  # row within the chunk
                nc.vector.tensor_copy(
                    out=AP(osh, hl * W, [[CHUNK * W, ND], [1, W]]),
                    in_=AP(tph, 0, [[W, ND], [1, W]]),
                )
            ostg_by_chunk[ci][1].append(b)
            if len(ostg_by_chunk[ci][1]) == nblocks:
                # chunk complete: one store, (dx,dy) -> d=9*dy+dx via AP dims
                osh = _th(ostg_by_chunk[ci][0])
                nc.sync.dma_start(
                    out=AP(
                        oh,
                        ci * CHUNK * W,
                        [[H * W, D], [D * H * W, D], [1, CHUNK * W]],
                    ),
                    in_=AP(osh, 0, [[CHUNK * W, ND], [1, CHUNK * W]]),
                )
                del ostg_by_chunk[ci]

        def emit_loads(ci):
            h0 = ci * CHUNK
            x1c = x_pool.tile([C, CHUNK * W], BF16, tag="x1c", name="x1c")
            x1h = _th(x1c)
            nc.gpsimd.dma_start(
                out=x1c[:],
                in_=AP(_th(x1), h0 * W, [[H * W, C], [1, CHUNK * W]]),
            )

            x2c = x_pool.tile([C, HC * W2], BF16, tag="x2c", name="x2c")
            x2h = _th(x2c)
            # zero the W-pad columns (4 left + 4 right per row)
            nc.gpsimd.memset(AP(x2h, 0, [[HC * W2, C], [W2, HC], [1, PAD]]), 0.0)
            nc.gpsimd.memset(
                AP(x2h, PAD + W, [[HC * W2, C], [W2, HC], [1, PAD]]), 0.0
            )
            lo = max(h0 - PAD, 0)
            hi = min(h0 + CHUNK + PAD, H)
            if lo > h0 - PAD:  # first chunk: zero top halo rows
                nc.gpsimd.memset(
                    AP(x2h, PAD, [[HC * W2, C], [W2, lo - (h0 - PAD)], [1, W]]), 0.0
                )
            if hi < h0 + CHUNK + PAD:  # last chunk: zero bottom halo rows
                r0 = hi - (h0 - PAD)
                nc.gpsimd.memset(
                    AP(x2h, r0 * W2 + PAD, [[HC * W2, C], [W2, HC - r0], [1, W]]),
                    0.0,
                )
            nc.gpsimd.dma_start(
                out=AP(
                    x2h,
                    (lo - (h0 - PAD)) * W2 + PAD,
                    [[HC * W2, C], [W2, hi - lo], [1, W]],
                ),
                in_=AP(_th(x2), lo * W, [[H * W, C], [1, (hi - lo) * W]]),
            )
            return x1h, x2h

        loaded = {0: emit_loads(0)}

        for ci in range(nchunks):
            h0 = ci * CHUNK
            x1h, x2h = loaded.pop(ci)

            ostg_by_chunk[ci] = [
                o_pool.tile([128, CHUNK * W], F32, tag="ostg", name="ostg"),
                [],
            ]

            for b in range(nblocks):
                h0b = h0 + b * HB
                # per-(block, wtile) fp16 band staging: [pixel, hh*360 + 9j+dy]
                stgs = [
                    stg_pool.tile([128, HB * BAND], F16, tag="stg", name=f"stg{wt}")
                    for wt in range(len(wtiles))
                ]
                for hh in range(HB):
                    h = h0b + hh
                    r = h - h0  # x1 row within chunk; x2 row base = r (top halo)
                    for wt, (w0, mw) in enumerate(wtiles):
                        nstrip = mw // 32
                        ps = ps_pool.tile([128, 512], F32, tag="band")
                        ph = _th(ps)
                        for s in range(nstrip):
                            wl = w0 + 32 * s
                            # moving contiguous (dy outer, j inner); psum OUT
                            # AP strided so the band lands already dy-minor:
                            # ps[m, 9j+dy]
                            nc.tensor.matmul(
                                AP(ph, 32 * s * 512, [[512, 32], [1, D], [D, WIN]]),
                                AP(x1h, r * W + wl, [[CHUNK * W, C], [1, 32]]),
                                AP(x2h, r * W2 + wl, [[HC * W2, C], [W2, D], [1, WIN]]),
                                start=True,
                                stop=True,
                                tile_position=(0, 32 * s),
                            )
                        # leaky eviction fp32 psum -> fp16 band, contiguous,
                        # one ACT op: stg[p, hh*360+t] = leaky_0.1(ps[p,t]/C)
                        sth = _th(stgs[wt])
                        nc.scalar.activation(
                            out=AP(
                                sth, hh * BAND, [[HB * BAND, mw], [1, BAND]]
                            ),
                            in_=AP(ph, 0, [[512, mw], [1, BAND]]),
                            func=mybir.ActivationFunctionType.Prelu,
                            scale=1.0 / C,
                            alpha=0.1,
                        )
                # band stores: one per wtile, contiguous 8640B per pixel
                for wt, (w0, mw) in enumerate(wtiles):
                    sth = _th(stgs[wt])
                    nc.sync.dma_start(
                        out=AP(
                            sh,
                            (w0 * H + h0b) * BAND,
                            [[H * BAND, mw], [1, HB * BAND]],
                        ),
                        in_=AP(sth, 0, [[HB * BAND, mw], [1, HB * BAND]]),
                    )
                # gathers for this block: per strip, HB h rows, diagonal on
                # DRAM (stride H*BAND+9); issue alternates HWDGE rings
                gbufs = []
                sg = 0
                for wt, (w0, mw) in enumerate(wtiles):
                    gbuf = g_pool.tile([128, HB * ND], F16, tag=f"gb{wt}")
                    gh = _th(gbuf)
                    for s in range(mw // 32):
                        wl = w0 + 32 * s
                        eng = nc.sync if sg % 2 == 0 else nc.scalar
                        sg += 1
                        eng.dma_start(
                            out=AP(
                                gh,
                                (32 * s) * (HB * ND),
                                [[HB * ND, 32], [ND, HB], [1, ND]],
                            ),
                            in_=AP(
                                sh,
                                (wl * H + h0b) * BAND,
                                [[H * BAND + D, 32], [BAND, HB], [1, ND]],
                            ),
                        )
                    gbufs.append(gbuf)
                if b == 0 and ci + 1 < nchunks:
                    # prefetch next chunk's inputs while this one computes
                    loaded[ci + 1] = emit_loads(ci + 1)
                if pending is not None:
                    flush_block(pending)
                pending = (ci, b, gbufs, h0b)

        flush_block(pending)

    return nc


_COMPILED = {}


def _build():
    key = "cv"
    if key not in _COMPILED:
        nc = bass.Bass()
        build_kernel(nc)
        _split_sync_waits(nc)
        _COMPILED[key] = nc
    return _COMPILED[key]


def kernel(**inputs) -> np.ndarray:
    x1 = np.asarray(inputs["x1"], dtype=np.float32)
    x2 = np.asarray(inputs["x2"], dtype=np.float32)
    B = x1.shape[0]
    nc = _build()
    core_ids = list(range(8))
    in_maps = [
        {"x1": np.ascontiguousarray(x1[b]), "x2": np.ascontiguousarray(x2[b])}
        for b in range(B)
    ]
    res = run_bass_kernel_spmd(nc, in_maps, core_ids)
    return np.stack([np.asarray(res.results[b]["out"]) for b in range(B)], axis=0)


# revision 27
# speedup vs baseline: 3.9218x; 1.1232x over previous
"""CostVolume (9x9 correlation window + leaky_relu) for trn2, 8 NeuronCores.

Problem: x1, x2: [B=8, C=128, H=96, W=320] fp32
  out[b, 9*dy+dx, h, w] =
      leaky_relu(mean_c(x1[b,c,h,w] * x2pad[b,c,h+dy,w+dx]), 0.1)
with x2 zero-padded by 4 on both spatial axes.

Sharding: data-parallel over batch - one batch per core, SPMD over 8 cores.

Per-core pipeline (single TileContext, h-chunk pipelined):
  - inputs stream HBM->SBUF via gpsimd cast-DMA straight to bf16 (the cast is
    free in the SDMA datapath; fp32 never lands in SBUF).
  - channel contraction on TensorE as bf16 band matmuls: per (h, 32-px strip)
    one matmul with the 9 dy-window rows batched into a single 360-wide moving
    AP; 4 strips pack the 128x128 PE array via tile_position col groups into
    one PSUM bank [128, 360].
  - eviction on DVE: one scalar_tensor_tensor max(0.1*x, x) (= unscaled leaky)
    converting PSUM fp32 -> SBUF fp16, relayouting dy-minor so each pixel's
    window values are contiguous (t = 9*j + dy).
  - deskew (extract each pixel's 81 diagonal values from the band) via a DRAM
    fp16 scratch: band store is contiguous; the per-pixel diagonal is an
    affine gather on the DRAM side (stride 369 = 360+9), batched over 8 h rows.
  - PE transposes [px, 81] -> [81, px] (fp16 identity), ACT copy applies the
    1/128 mean scale and converts to fp32, one big store per 24-row chunk with
    the (dx,dy)->d=9*dy+dx permutation absorbed in the store AP.
"""

from contextlib import ExitStack

import numpy as np

import concourse.bass as bass
import concourse.mybir as mybir
import concourse.tile as tile
from concourse import masks
from concourse.ap import AP
from concourse.bass_utils import run_bass_kernel_spmd

F32 = mybir.dt.float32
F16 = mybir.dt.float16
BF16 = mybir.dt.bfloat16
C = 128
PAD = 4
D = 9
ND = D * D  # 81
WIN = 40  # 32 + 8: moving window per strip
BAND = D * WIN  # 360 band values per pixel in scratch

# The walrus build in this toolchain rejects instructions carrying more than
# one sync wait ("Too many sync wait commands"). Split: any instruction with
# >1 on_wait gets preceding same-engine NoOps carrying the extra waits
# (engine streams execute in order, so the gating is identical).
_MAX_WAITS = 1


def _split_sync_waits(nc):
    for bbname, bassbb in nc.bb_map.items():
        bb = bassbb.bb
        insts = list(bb.instructions)
        out = []
        changed = False
        for inst in insts:
            si = getattr(inst, "sync_info", None)
            waits = list(si.on_wait) if (si is not None and si.on_wait) else []
            if len(waits) > _MAX_WAITS:
                changed = True
                spill, keep = waits[:-_MAX_WAITS], waits[-_MAX_WAITS:]
                for k in range(0, len(spill), _MAX_WAITS):
                    chunk = spill[k : k + _MAX_WAITS]
                    out.append(
                        mybir.InstNoOp(
                            name=f"I-waitsplit-{nc.next_id()}",
                            engine=inst.engine,
                            ins=[],
                            outs=[],
                            sync_info=mybir.SyncInfo(on_wait=chunk, on_update=[]),
                        )
                    )
                si.on_wait = keep
            out.append(inst)
        if changed:
            try:
                bb.instructions[:] = out
            except TypeError:
                while len(bb.instructions):
                    bb.instructions.pop()
                for i in out:
                    bb.add_instruction(i)


def _th(t):
    return t.tensor if isinstance(t, AP) else t


def build_kernel(nc: bass.Bass, H=96, W=320, CHUNK=24, HB=12):
    assert H % CHUNK == 0 and CHUNK % HB == 0
    x1 = nc.declare_dram_parameter("x1", [C, H, W], F32, isOutput=False)
    x2 = nc.declare_dram_parameter("x2", [C, H, W], F32, isOutput=False)
    out = nc.declare_dram_parameter("out", [ND, H, W], F32, isOutput=True)

    # fp16 band scratch, w-major: addr = (w*H + h)*BAND + 9*j + dy
    # (w-major makes the band store one contiguous 8640B run per pixel-block)
    scratch = nc.dram_tensor("scratch", [H * W * BAND], F16)
    sh = _th(scratch)
    oh = _th(out)

    W2 = W + 2 * PAD
    HC = CHUNK + 2 * PAD
    # (w0, mw): 128/128/64 pixel column tiles; each holds mw//32 strips
    wtiles = []
    wleft, w0 = W, 0
    while wleft > 0:
        mw = min(128, wleft)
        wtiles.append((w0, mw))
        w0 += mw
        wleft -= mw
    nchunks = H // CHUNK
    nblocks = CHUNK // HB

    with tile.TileContext(nc) as tc, ExitStack() as ctx:
        const_pool = ctx.enter_context(tc.tile_pool(name="const", bufs=1))
        x_pool = ctx.enter_context(tc.tile_pool(name="x", bufs=2))
        stg_pool = ctx.enter_context(tc.tile_pool(name="stg", bufs=4))
        rl_pool = ctx.enter_context(tc.tile_pool(name="rl", bufs=2))
        g_pool = ctx.enter_context(tc.tile_pool(name="g", bufs=2))
        o_pool = ctx.enter_context(tc.tile_pool(name="o", bufs=2))
        ps_pool = ctx.enter_context(tc.tile_pool(name="ps", bufs=4, space="PSUM"))
        tp_pool = ctx.enter_context(tc.tile_pool(name="tp", bufs=3, space="PSUM"))

        ident = const_pool.tile([128, 128], F16)
        masks.make_identity(nc, ident[:])
        idh = _th(ident)

        # deferred per-block state: (ci, b, gbufs, h0b)
        pending = None
        ostg_by_chunk = {}

        def flush_block(blk):
            ci, b, gbufs, h0b = blk
            ostg, _ = ostg_by_chunk[ci]
            osh = _th(ostg)
            for hh in range(HB):
                tp = tp_pool.tile([128, W], F16, tag="tp")
                tph = _th(tp)
                for wt, (w0, mw) in enumerate(wtiles):
                    gh = _th(gbufs[wt])
                    nc.tensor.transpose(
                        AP(tph, w0, [[W, ND], [1, mw]]),
                        AP(gh, hh * ND, [[HB * ND, mw], [1, ND]]),
                        AP(idh, 0, [[128, mw], [1, mw]]),
                    )
                hl = (h0b - ci * CHUNK) + hh  # row within the chunk
                nc.vector.tensor_copy(
                    out=AP(osh, hl * W, [[CHUNK * W, ND], [1, W]]),
                    in_=AP(tph, 0, [[W, ND], [1, W]]),
                )
            ostg_by_chunk[ci][1].append(b)
            if len(ostg_by_chunk[ci][1]) == nblocks:
                # chunk complete: one store, (dx,dy) -> d=9*dy+dx via AP dims
                osh = _th(ostg_by_chunk[ci][0])
                nc.sync.dma_start(
                    out=AP(
                        oh,
                        ci * CHUNK * W,
                        [[H * W, D], [D * H * W, D], [1, CHUNK * W]],
                    ),
                    in_=AP(osh, 0, [[CHUNK * W, ND], [1, CHUNK * W]]),
                )
                del ostg_by_chunk[ci]

        def emit_loads(ci):
            h0 = ci * CHUNK
            x1c = x_pool.tile([C, CHUNK * W], BF16, tag="x1c", name="x1c")
            x1h = _th(x1c)
            x2c = x_pool.tile([C, HC * W2], BF16, tag="x2c", name="x2c")
            x2h = _th(x2c)

            def load_x1(a, b1):
                nc.gpsimd.dma_start(
                    out=AP(x1h, a * W, [[CHUNK * W, C], [1, (b1 - a) * W]]),
                    in_=AP(_th(x1), (h0 + a) * W, [[H * W, C], [1, (b1 - a) * W]]),
                )

            def load_x2(a, b2):
                nc.gpsimd.dma_start(
                    out=AP(
                        x2h,
                        (a - (h0 - PAD)) * W2 + PAD,
                        [[HC * W2, C], [W2, b2 - a], [1, W]],
                    ),
                    in_=AP(_th(x2), a * W, [[H * W, C], [1, (b2 - a) * W]]),
                )

            # zero the W-pad columns (4 left + 4 right per row)
            nc.gpsimd.memset(AP(x2h, 0, [[HC * W2, C], [W2, HC], [1, PAD]]), 0.0)
            nc.gpsimd.memset(
                AP(x2h, PAD + W, [[HC * W2, C], [W2, HC], [1, PAD]]), 0.0
            )
            lo = max(h0 - PAD, 0)
            hi = min(h0 + CHUNK + PAD, H)
            if lo > h0 - PAD:  # first chunk: zero top halo rows
                nc.gpsimd.memset(
                    AP(x2h, PAD, [[HC * W2, C], [W2, lo - (h0 - PAD)], [1, W]]), 0.0
                )
            if hi < h0 + CHUNK + PAD:  # last chunk: zero bottom halo rows
                r0 = hi - (h0 - PAD)
                nc.gpsimd.memset(
                    AP(x2h, r0 * W2 + PAD, [[HC * W2, C], [W2, HC - r0], [1, W]]),
                    0.0,
                )
            if ci == 0:
                # split the cold-start chunk so block 0 starts after half
                mid = h0 + HB + PAD
                load_x1(0, HB)
                load_x2(lo, mid)
                load_x1(HB, CHUNK)
                load_x2(mid, hi)
            else:
                load_x1(0, CHUNK)
                load_x2(lo, hi)
            return x1h, x2h

        loaded = {0: emit_loads(0)}

        for ci in range(nchunks):
            h0 = ci * CHUNK
            x1h, x2h = loaded.pop(ci)

            ostg_by_chunk[ci] = [
                o_pool.tile([128, CHUNK * W], F32, tag="ostg", name="ostg"),
                [],
            ]

            for b in range(nblocks):
                h0b = h0 + b * HB
                # per-(block, wtile) fp16 band staging: [pixel, hh*360 + 9j+dy]
                stgs = [
                    stg_pool.tile([128, HB * BAND], F16, tag="stg", name=f"stg{wt}")
                    for wt in range(len(wtiles))
                ]
                for hh in range(HB):
                    h = h0b + hh
                    r = h - h0  # x1 row within chunk; x2 row base = r (top halo)
                    for wt, (w0, mw) in enumerate(wtiles):
                        nstrip = mw // 32
                        ps = ps_pool.tile([128, 512], F32, tag="band")
                        ph = _th(ps)
                        for s in range(nstrip):
                            wl = w0 + 32 * s
                            # moving contiguous (dy outer, j inner); psum OUT
                            # AP strided so the band lands already dy-minor:
                            # ps[m, 9j+dy]
                            nc.tensor.matmul(
                                AP(ph, 32 * s * 512, [[512, 32], [1, D], [D, WIN]]),
                                AP(x1h, r * W + wl, [[CHUNK * W, C], [1, 32]]),
                                AP(x2h, r * W2 + wl, [[HC * W2, C], [W2, D], [1, WIN]]),
                                start=True,
                                stop=True,
                                tile_position=(0, 32 * s),
                            )
                        # leaky eviction fp32 psum -> fp16 band, contiguous:
                        # stg[p, hh*360+t] = leaky_0.1(ps[p,t]/C).
                        # ACT Prelu mostly; wt1 alternates to DVE (2-op
                        # relu+combine) to balance engine load.
                        sth = _th(stgs[wt])
                        if wt == 1 and hh % 2 == 1:
                            relu_t = rl_pool.tile(
                                [128, BAND], F16, tag="relu_t", name="relu_t"
                            )
                            rth = _th(relu_t)
                            nc.vector.tensor_scalar(
                                out=AP(rth, 0, [[BAND, mw], [1, BAND]]),
                                in0=AP(ph, 0, [[512, mw], [1, BAND]]),
                                scalar1=0.9 / C,
                                scalar2=0.0,
                                op0=mybir.AluOpType.mult,
                                op1=mybir.AluOpType.max,
                            )
                            nc.vector.scalar_tensor_tensor(
                                out=AP(
                                    sth, hh * BAND, [[HB * BAND, mw], [1, BAND]]
                                ),
                                in0=AP(ph, 0, [[512, mw], [1, BAND]]),
                                scalar=0.1 / C,
                                in1=AP(rth, 0, [[BAND, mw], [1, BAND]]),
                                op0=mybir.AluOpType.mult,
                                op1=mybir.AluOpType.add,
                            )
                        else:
                            nc.scalar.activation(
                                out=AP(
                                    sth, hh * BAND, [[HB * BAND, mw], [1, BAND]]
                                ),
                                in_=AP(ph, 0, [[512, mw], [1, BAND]]),
                                func=mybir.ActivationFunctionType.Prelu,
                                scale=1.0 / C,
                                alpha=0.1,
                            )
                # band stores: strips 1.. only (strip 0 deskews on-chip),
                # contiguous 8640B per pixel
                for wt, (w0, mw) in enumerate(wtiles):
                    sth = _th(stgs[wt])
                    nc.sync.dma_start(
                        out=AP(
                            sh,
                            ((w0 + 32) * H + h0b) * BAND,
                            [[H * BAND, mw - 32], [1, HB * BAND]],
                        ),
                        in_=AP(
                            sth,
                            32 * HB * BAND,
                            [[HB * BAND, mw - 32], [1, HB * BAND]],
                        ),
                    )
                # deskew. strip 0 of each wtile: direct SBUF->SBUF diagonal
                # (single partition-crossing dim at offset 0). strips 1..:
                # DRAM diagonal gather (stride H*BAND+9), HB rows per DMA.
                gbufs = []
                for wt, (w0, mw) in enumerate(wtiles):
                    gbuf = g_pool.tile([128, HB * ND], F16, tag=f"gb{wt}")
                    gh = _th(gbuf)
                    nc.scalar.dma_start(
                        out=AP(gh, 0, [[HB * ND, 32], [ND, HB], [1, ND]]),
                        in_=AP(
                            _th(stgs[wt]),
                            0,
                            [[HB * BAND + D, 32], [BAND, HB], [1, ND]],
                        ),
                    )
                    for s in range(1, mw // 32):
                        wl = w0 + 32 * s
                        nc.sync.dma_start(
                            out=AP(
                                gh,
                                (32 * s) * (HB * ND),
                                [[HB * ND, 32], [ND, HB], [1, ND]],
                            ),
                            in_=AP(
                                sh,
                                (wl * H + h0b) * BAND,
                                [[H * BAND + D, 32], [BAND, HB], [1, ND]],
                            ),
                        )
                    gbufs.append(gbuf)
                if b == 0 and ci + 1 < nchunks:
                    # prefetch next chunk's inputs while this one computes
                    loaded[ci + 1] = emit_loads(ci + 1)
                if pending is not None:
                    flush_block(pending)
                pending = (ci, b, gbufs, h0b)

        flush_block(pending)

    return nc


_COMPILED = {}


def _build():
    key = "cv"
    if key not in _COMPILED:
        nc = bass.Bass()
        build_kernel(nc)
        _split_sync_waits(nc)
        _COMPILED[key] = nc
    return _COMPILED[key]


def kernel(**inputs) -> np.ndarray:
    x1 = np.asarray(inputs["x1"], dtype=np.float32)
    x2 = np.asarray(inputs["x2"], dtype=np.float32)
    B = x1.shape[0]
    nc = _build()
    core_ids = list(range(8))
    in_maps = [
        {"x1": np.ascontiguousarray(x1[b]), "x2": np.ascontiguousarray(x2[b])}
        for b in range(B)
    ]
    res = run_bass_kernel_spmd(nc, in_maps, core_ids)
    return np.stack([np.asarray(res.results[b]["out"]) for b in range(B)], axis=0)
